# revision 1
# baseline (speedup 1.0000x reference)
"""Cross-attention kernel for 8 Trainium2 NeuronCores.

Sharding: 8 cores = 4 batches x 2 head-groups (6 heads each).
Per core (b, hg), with all activations pre-transposed on host:
  qT = (Wq_hg*scale).T' @ xqT   [384, 2048]   (weights column-split)
  kT = Wk_hg' @ xkT             [384, 2048]
  v  = xvT' @ Wv_hg.T           [2048, 384]  (+ ones column per head)
  per head h: lt = k_h qT_h     [2048k, 2048q] (logits transposed)
              p  = exp(lt)      (no max-subtraction: logits are O(1))
              [x; d] = [v_h|1].T @ p   -> x rows 0..63, denominators row 64
              xn = x * (1/d)    (partition-broadcast of 1/d)
  outT_partial = Wo_hg.T' @ xn  [768, 2048]
Host: out[b] = (partial[2b] + partial[2b+1]).T + bo.

All matmuls bf16 inputs with fp32 PSUM accumulation.
"""

import sys

import numpy as np

for _p in ("/opt/trn_rl_repo",):
    if _p not in sys.path:
        sys.path.insert(0, _p)

B, NQ, NK, C = 4, 2048, 2048, 768
H, DH = 12, 64
HPC, HB = 6, 384  # heads per core, head-block width
P = 128
KT = C // P  # 6 contraction k-tiles for projections
QCH = 512  # query-chunk width
NCH = NQ // QCH  # 4 query chunks
NKT = NK // P  # 16 key tiles
SCALE = DH**-0.5  # folded into Wq on host (exactly 0.125)
VW = DH + 1  # v block width per head incl. ones column

_prog = None


def _build():
    from contextlib import ExitStack

    import concourse.bass as bass
    import concourse.tile as tile
    from concourse import mybir
    from concourse.bacc import Bacc

    f32 = mybir.dt.float32
    bf16 = mybir.dt.bfloat16
    EXP = mybir.ActivationFunctionType.Exp

    nc = Bacc()
    xq_d = nc.declare_dram_parameter("xq", [C, NQ], bf16, isOutput=False)
    xk_d = nc.declare_dram_parameter("xk", [C, NK], bf16, isOutput=False)
    xv_d = nc.declare_dram_parameter("xv", [C, NK], bf16, isOutput=False)
    wq_d = nc.declare_dram_parameter("wq", [C, HB], bf16, isOutput=False)
    wk_d = nc.declare_dram_parameter("wk", [C, HB], bf16, isOutput=False)
    wv_d = nc.declare_dram_parameter("wv", [C, HB], bf16, isOutput=False)
    wo_d = nc.declare_dram_parameter("wo", [HB, C], bf16, isOutput=False)
    out_d = nc.declare_dram_parameter("out", [C, NQ], f32, isOutput=True)

    with tile.TileContext(nc) as tc, ExitStack() as ctx:
        const = ctx.enter_context(tc.tile_pool(name="const", bufs=1))
        xin = ctx.enter_context(tc.tile_pool(name="xin", bufs=KT))
        qk = ctx.enter_context(tc.tile_pool(name="qk", bufs=3))
        pp = ctx.enter_context(tc.tile_pool(name="pp", bufs=16))
        xnp = ctx.enter_context(tc.tile_pool(name="xnp", bufs=3))
        small = ctx.enter_context(tc.tile_pool(name="small", bufs=2))
        ost = ctx.enter_context(tc.tile_pool(name="ost", bufs=3))
        drp = ctx.enter_context(tc.tile_pool(name="drp", bufs=2, space="DRAM"))
        proj_ps = ctx.enter_context(tc.tile_pool(name="proj_ps", bufs=2, space="PSUM"))
        lt_ps = ctx.enter_context(tc.tile_pool(name="lt_ps", bufs=2, space="PSUM"))
        x_ps = ctx.enter_context(tc.tile_pool(name="x_ps", bufs=2, space="PSUM"))

        # ---- weights + inputs to SBUF
        wq_s = const.tile([P, KT, HB], bf16, tag="wq")
        wk_s = const.tile([P, KT, HB], bf16, tag="wk")
        wv_s = const.tile([P, KT, HB], bf16, tag="wv")
        wo_s = const.tile([P, HB // P, C], bf16, tag="wo")
        nc.sync.dma_start(out=wq_s, in_=wq_d.rearrange("(k p) m -> p k m", p=P))
        nc.sync.dma_start(out=wk_s, in_=wk_d.rearrange("(k p) m -> p k m", p=P))
        nc.sync.dma_start(out=wv_s, in_=wv_d.rearrange("(k p) m -> p k m", p=P))
        nc.sync.dma_start(out=wo_s, in_=wo_d.rearrange("(k p) m -> p k m", p=P))

        xq_t, xk_t, xv_t = [], [], []
        for k in range(KT):
            for name, dram, lst in (
                ("xq", xq_d, xq_t),
                ("xk", xk_d, xk_t),
                ("xv", xv_d, xv_t),
            ):
                t = xin.tile([P, NQ], bf16, tag=name, name=f"{name}_{k}")
                nc.sync.dma_start(
                    out=t, in_=dram.rearrange("(k p) m -> p k m", p=P)[:, k, :]
                )
                lst.append(t)

        # v with a ones column per head: [128, kt, head, 65]
        v_s = const.tile([P, NKT, HPC, VW], bf16, tag="v")
        nc.vector.memset(v_s[:, :, :, DH : DH + 1], 1.0)

        qT_t = [qk.tile([P, NQ], bf16, tag="qT", name=f"qT{i}") for i in range(3)]
        kT_t = [qk.tile([P, NK], bf16, tag="kT", name=f"kT{i}") for i in range(3)]

        def proj_qk(w_s, src, dst, mt):
            for j4 in range(NCH):
                ps = proj_ps.tile([P, QCH], f32, tag="proj")
                for k in range(KT):
                    nc.tensor.matmul(
                        ps,
                        w_s[:, k, mt * P : (mt + 1) * P],
                        src[k][:, j4 * QCH : (j4 + 1) * QCH],
                        start=(k == 0),
                        stop=(k == KT - 1),
                    )
                nc.vector.tensor_copy(dst[:, j4 * QCH : (j4 + 1) * QCH], ps)

        # projections for head-pair 0 first so ACT can start early
        proj_qk(wq_s, xq_t, qT_t[0], 0)
        proj_qk(wk_s, xk_t, kT_t[0], 0)
        for mt in (1, 2):
            proj_qk(wq_s, xq_t, qT_t[mt], mt)
            proj_qk(wk_s, xk_t, kT_t[mt], mt)

        # v projection: [2048, 384] natural layout, strided into v_s
        for kt in range(NKT):
            ps = proj_ps.tile([P, HB], f32, tag="proj")
            for k in range(KT):
                nc.tensor.matmul(
                    ps,
                    xv_t[k][:, kt * P : (kt + 1) * P],
                    wv_s[:, k, :],
                    start=(k == 0),
                    stop=(k == KT - 1),
                )
            nc.vector.tensor_copy(
                v_s[:, kt, :, 0:DH], ps.rearrange("p (h m) -> p h m", m=DH)
            )

        xn_t = [xnp.tile([P, NQ], bf16, tag="xn", name=f"xn{i}") for i in range(3)]

        for j4 in range(NCH):
            qsl = slice(j4 * QCH, (j4 + 1) * QCH)
            for p3 in range(3):
                rows = (slice(0, DH), slice(DH, 2 * DH))
                p_tiles = ([], [])
                # QK^T row-packed across the head pair + exp
                for ktp in range(NKT // 2):
                    lts = [lt_ps.tile([P, 2 * QCH], f32, tag="lt", name=f"lt{i}") for i in range(2)]
                    for u in range(2):
                        kt = 2 * ktp + u
                        for hh in range(2):
                            nc.tensor.matmul(
                                lts[hh][:, u * QCH : (u + 1) * QCH],
                                kT_t[p3][rows[hh], kt * P : (kt + 1) * P],
                                qT_t[p3][rows[hh], qsl],
                                start=True,
                                stop=True,
                            )
                    for hh in range(2):
                        pt = pp.tile([P, 2 * QCH], bf16, tag="p")
                        nc.scalar.activation(pt, lts[hh], EXP)
                        p_tiles[hh].append(pt)
                # AV with ones-augmented v: rows 0..63 = x, row 64 = denom
                for hh in range(2):
                    h = 2 * p3 + hh
                    xps = x_ps.tile([DH + 1, QCH], f32, tag="x")
                    for kt in range(NKT):
                        pt = p_tiles[hh][kt // 2][:, (kt % 2) * QCH : (kt % 2 + 1) * QCH]
                        nc.tensor.matmul(
                            xps,
                            v_s[:, kt, h, :],
                            pt,
                            start=(kt == 0),
                            stop=(kt == NKT - 1),
                        )
                    r = small.tile([1, QCH], f32, tag="r")
                    nc.vector.reciprocal(r, xps[DH : DH + 1, :])
                    rd = drp.tile([1, QCH], f32, tag="rd")
                    nc.sync.dma_start(out=rd, in_=r)
                    rb = small.tile([DH, QCH], f32, tag="rb")
                    nc.gpsimd.dma_start(out=rb, in_=rd.to_broadcast([DH, QCH]))
                    if hh == 0:
                        nc.vector.tensor_mul(xn_t[p3][0:DH, qsl], xps[0:DH, :], rb)
                    else:
                        tmp = small.tile([DH, QCH], bf16, tag="tmp")
                        nc.vector.tensor_mul(tmp, xps[0:DH, :], rb)
                        nc.sync.dma_start(out=xn_t[p3][DH : 2 * DH, qsl], in_=tmp)
            # output projection for this chunk
            for mt in range(C // P):
                ps = proj_ps.tile([P, QCH], f32, tag="proj")
                for k3 in range(HB // P):
                    nc.tensor.matmul(
                        ps,
                        wo_s[:, k3, mt * P : (mt + 1) * P],
                        xn_t[k3][:, qsl],
                        start=(k3 == 0),
                        stop=(k3 == HB // P - 1),
                    )
                o = ost.tile([P, QCH], f32, tag="o")
                nc.vector.tensor_copy(o, ps)
                nc.sync.dma_start(out=out_d[mt * P : (mt + 1) * P, qsl], in_=o)

    nc.finalize()
    return nc


def _get_prog():
    global _prog
    if _prog is None:
        _prog = _build()
    return _prog


def _shard_inputs(query, key, value, Wq, Wk, Wv, Wo):
    from ml_dtypes import bfloat16

    in_maps = []
    for core in range(8):
        b, hg = core // 2, core % 2
        sl = slice(hg * HB, (hg + 1) * HB)
        in_maps.append(
            {
                "xq": np.ascontiguousarray(query[b].T).astype(bfloat16),
                "xk": np.ascontiguousarray(key[b].T).astype(bfloat16),
                "xv": np.ascontiguousarray(value[b].T).astype(bfloat16),
                "wq": np.ascontiguousarray((Wq[sl, :] * SCALE).T).astype(bfloat16),
                "wk": np.ascontiguousarray(Wk[sl, :].T).astype(bfloat16),
                "wv": np.ascontiguousarray(Wv[sl, :].T).astype(bfloat16),
                "wo": np.ascontiguousarray(Wo[:, sl].T).astype(bfloat16),
            }
        )
    return in_maps


def kernel(query, key, value, Wq, Wk, Wv, Wo, bo):
    query, key, value = np.asarray(query), np.asarray(key), np.asarray(value)
    Wq, Wk, Wv, Wo = np.asarray(Wq), np.asarray(Wk), np.asarray(Wv), np.asarray(Wo)
    bo = np.asarray(bo).astype(np.float32)

    from concourse.bass_utils import run_bass_kernel_spmd

    nc = _get_prog()
    in_maps = _shard_inputs(query, key, value, Wq, Wk, Wv, Wo)
    res = run_bass_kernel_spmd(nc, in_maps, list(range(8))).results

    out = np.empty((B, NQ, C), np.float32)
    for b in range(B):
        acc = res[2 * b]["out"].astype(np.float32) + res[2 * b + 1]["out"].astype(
            np.float32
        )
        out[b] = acc.T + bo[None, :]
    return out



# revision 19
# speedup vs baseline: 1.3780x; 1.3780x over previous
"""Cross-attention kernel for 8 Trainium2 NeuronCores.

Sharding: 8 cores = 4 batches x 2 head-groups (6 heads each).
Per core (b, hg), with all activations pre-transposed on host:
  qT = (Wq_hg*scale).T' @ xqT   [384, 2048]   (weights column-split)
  kT = Wk_hg' @ xkT             [384, 2048]
  v  = xvT' @ Wv_hg.T           [2048, 384]  (+ ones column per head)
  per head h: lt = k_h qT_h     [2048k, 2048q] (logits transposed, bf16 PSUM)
              p  = exp(lt)      (no max-subtraction: logits are O(1))
              [x; d] = [v_h|1].T @ p   -> x rows 0..63, denominators row 64
              xn = x * (1/d)    (partition-broadcast of 1/d via DRAM DMA)
  outT_partial = Wo_hg.T' @ xn  [768, 2048]  (DMA'd straight from PSUM)
Host: out[b] = (partial[2b] + partial[2b+1]).T + bo.

Pipeline: per (q-chunk, head-pair) iteration, QK pairs run row-tiled
concurrently (K=64 heads at partitions 0:64/64:128), exp consumes
[128,2048] bf16 PSUM tiles, AV trails exp by 2 kt-pairs, and the
projection chains are dripped into the PE stream to keep HAM warm.
"""

import sys

import numpy as np

for _p in ("/opt/trn_rl_repo",):
    if _p not in sys.path:
        sys.path.insert(0, _p)

B, NQ, NK, C = 4, 2048, 2048, 768
H, DH = 12, 64
HPC, HB = 6, 384  # heads per core, head-block width
P = 128
KT = C // P  # 6 contraction k-tiles for projections
QCH = 512  # query-chunk width
NCH = NQ // QCH  # 4 query chunks
NKT = NK // P  # 16 key tiles
SCALE = DH**-0.5  # folded into Wq on host (exactly 0.125)
VW = DH + 1  # v block width per head incl. ones column

_prog = None


def _build():
    from contextlib import ExitStack

    import concourse.bass as bass
    import concourse.tile as tile
    from concourse import mybir
    from concourse.bacc import Bacc

    f32 = mybir.dt.float32
    bf16 = mybir.dt.bfloat16
    EXP = mybir.ActivationFunctionType.Exp

    nc = Bacc()
    xq_d = nc.declare_dram_parameter("xq", [C, NQ], bf16, isOutput=False)
    xk_d = nc.declare_dram_parameter("xk", [C, NK], bf16, isOutput=False)
    xv_d = nc.declare_dram_parameter("xv", [C, NK], bf16, isOutput=False)
    wq_d = nc.declare_dram_parameter("wq", [C, HB], bf16, isOutput=False)
    wk_d = nc.declare_dram_parameter("wk", [C, HB], bf16, isOutput=False)
    wv_d = nc.declare_dram_parameter("wv", [C, HB], bf16, isOutput=False)
    wo_d = nc.declare_dram_parameter("wo", [HB, C], bf16, isOutput=False)
    out_d = nc.declare_dram_parameter("out", [C, NQ], bf16, isOutput=True)

    with tile.TileContext(nc) as tc, ExitStack() as ctx:
        const = ctx.enter_context(tc.tile_pool(name="const", bufs=1))
        xin = ctx.enter_context(tc.tile_pool(name="xin", bufs=KT))
        qk = ctx.enter_context(tc.tile_pool(name="qk", bufs=3))
        pp = ctx.enter_context(tc.tile_pool(name="pp", bufs=6))
        xnp = ctx.enter_context(tc.tile_pool(name="xnp", bufs=3))
        small = ctx.enter_context(tc.tile_pool(name="small", bufs=4))
        brd = ctx.enter_context(tc.tile_pool(name="brd", bufs=4))
        ost = ctx.enter_context(tc.tile_pool(name="ost", bufs=3))
        drp = ctx.enter_context(tc.tile_pool(name="drp", bufs=4, space="DRAM"))
        proj_ps = ctx.enter_context(tc.tile_pool(name="proj_ps", bufs=2, space="PSUM"))
        lt_ps = ctx.enter_context(tc.tile_pool(name="lt_ps", bufs=2, space="PSUM"))
        x_ps = ctx.enter_context(tc.tile_pool(name="x_ps", bufs=2, space="PSUM"))

        # ---- weights + inputs to SBUF
        wq_s = const.tile([P, KT, HB], bf16, tag="wq")
        wk_s = const.tile([P, KT, HB], bf16, tag="wk")
        wv_s = const.tile([P, KT, HB], bf16, tag="wv")
        wo_s = const.tile([P, HB // P, C], bf16, tag="wo")
        nc.sync.dma_start(out=wq_s, in_=wq_d.rearrange("(k p) m -> p k m", p=P))
        nc.sync.dma_start(out=wk_s, in_=wk_d.rearrange("(k p) m -> p k m", p=P))
        nc.sync.dma_start(out=wv_s, in_=wv_d.rearrange("(k p) m -> p k m", p=P))
        nc.sync.dma_start(out=wo_s, in_=wo_d.rearrange("(k p) m -> p k m", p=P))

        xq_t, xk_t, xv_t = [], [], []
        for name, dram, lst in (
            ("xq", xq_d, xq_t),
            ("xk", xk_d, xk_t),
            ("xv", xv_d, xv_t),
        ):
            for k in range(KT):
                t = xin.tile([P, NQ], bf16, tag=name, name=f"{name}_{k}")
                nc.sync.dma_start(
                    out=t, in_=dram.rearrange("(k p) m -> p k m", p=P)[:, k, :]
                )
                lst.append(t)

        # v with a ones column per head: [128, kt, head, 65]
        v_s = const.tile([P, NKT, HPC, VW], bf16, tag="v")
        nc.vector.memset(v_s[:, :, :, DH : DH + 1], 1.0)

        # Warm-up during the input-DMA window: ~10us of dummy matmuls gets
        # HAM to K=8/8 before real work, and a dummy exp absorbs the ~2.7us
        # ACT table load off the critical path.  Both depend only on the
        # (small, early) weight DMAs.
        warm_ps = lt_ps.tile([P, 2 * QCH], f32, tag="lt", name="warm")
        for i in range(28):
            nc.tensor.matmul(
                warm_ps[:, 0:HB],
                wq_s[:, i % KT, 0:P],
                wq_s[:, (i + 1) % KT, :],
                start=True,
                stop=True,
            )
        warm_act = small.tile([1, QCH], f32, tag="r0", name="warm_act")
        nc.scalar.activation(warm_act[:, 0:HB], warm_ps[0:1, 0:HB], EXP)

        qT_t = [qk.tile([P, NQ], bf16, tag="qT", name=f"qT{i}") for i in range(3)]
        kT_t = [qk.tile([P, NK], bf16, tag="kT", name=f"kT{i}") for i in range(3)]
        xn_t = [xnp.tile([P, NQ], bf16, tag="xn", name=f"xn{i}") for i in range(3)]

        # ---- deferred projection-chain emitters -------------------------
        def qk_chain(w_s, src, dst, mt, j4):
            ps = proj_ps.tile([P, QCH], f32, tag="proj")
            for k in range(KT):
                nc.tensor.matmul(
                    ps,
                    w_s[:, k, mt * P : (mt + 1) * P],
                    src[k][:, j4 * QCH : (j4 + 1) * QCH],
                    start=(k == 0),
                    stop=(k == KT - 1),
                )
            nc.vector.tensor_copy(dst[:, j4 * QCH : (j4 + 1) * QCH], ps)

        def v_chain(kt):
            ps = proj_ps.tile([P, QCH], f32, tag="proj")
            for k in range(KT):
                nc.tensor.matmul(
                    ps[:, 0:HB],
                    xv_t[k][:, kt * P : (kt + 1) * P],
                    wv_s[:, k, :],
                    start=(k == 0),
                    stop=(k == KT - 1),
                )
            nc.vector.tensor_copy(
                v_s[:, kt, :, 0:DH], ps[:, 0:HB].rearrange("p (h m) -> p h m", m=DH)
            )

        def oproj_chain(j4, mt):
            qsl = slice(j4 * QCH, (j4 + 1) * QCH)
            ps = proj_ps.tile([P, QCH], f32, tag="proj")
            for k3 in range(HB // P):
                nc.tensor.matmul(
                    ps,
                    wo_s[:, k3, mt * P : (mt + 1) * P],
                    xn_t[k3][:, qsl],
                    start=(k3 == 0),
                    stop=(k3 == HB // P - 1),
                )
            o = ost.tile([P, QCH], bf16, tag="o")
            nc.vector.tensor_copy(o, ps)
            nc.sync.dma_start(out=out_d[mt * P : (mt + 1) * P, qsl], in_=o)

        # drip schedule: iteration index -> list of thunks to interleave.
        # Iteration order is pair-outer: it = p3*4 + j4.  kT_t[p3] must be
        # FULLY projected (all key chunks) before iteration p3*4; qT_t[p3]
        # only needs q-chunk j4 by iteration p3*4+j4; v chains before the
        # AVs that use them; out-proj(j4) after iteration 8+j4.
        def q_chain(p3, j4):
            qk_chain(wq_s, xq_t, qT_t[p3], p3, j4)

        def k_chain(p3, kc):
            qk_chain(wk_s, xk_t, kT_t[p3], p3, kc)

        drip = {i: [] for i in range(12)}
        upfront = (
            [lambda kc=kc: k_chain(0, kc) for kc in range(4)]
            + [lambda: q_chain(0, 0)]
            + [lambda kt=kt: v_chain(kt) for kt in range(4)]
        )
        drip[0] = [lambda kt=kt: v_chain(kt) for kt in range(4, 16)] + [
            lambda: q_chain(0, 1)
        ]
        drip[1] = [
            lambda: q_chain(0, 2),
            lambda: k_chain(1, 0),
            lambda: k_chain(1, 1),
        ]
        drip[2] = [
            lambda: q_chain(0, 3),
            lambda: k_chain(1, 2),
            lambda: k_chain(1, 3),
        ]
        drip[3] = [lambda: q_chain(1, 0), lambda: k_chain(2, 0)]
        drip[4] = [lambda: q_chain(1, 1), lambda: k_chain(2, 1)]
        drip[5] = [lambda: q_chain(1, 2), lambda: k_chain(2, 2)]
        drip[6] = [lambda: q_chain(1, 3), lambda: k_chain(2, 3)]
        drip[7] = [lambda: q_chain(2, 0)]
        drip[8] = [lambda: q_chain(2, 1)]
        drip[9] = [lambda: q_chain(2, 2)] + [
            lambda mt=mt: oproj_chain(0, mt) for mt in range(C // P)
        ]
        drip[10] = [lambda: q_chain(2, 3)] + [
            lambda mt=mt: oproj_chain(1, mt) for mt in range(C // P)
        ]
        drip[11] = [lambda mt=mt: oproj_chain(2, mt) for mt in range(C // P)]
        for th in upfront:
            th()

        # ---- attention pipeline ----------------------------------------
        for it in range(12):
            p3, j4 = divmod(it, 4)
            qsl = slice(j4 * QCH, (j4 + 1) * QCH)
            rows = (slice(0, DH), slice(DH, 2 * DH))
            x_tiles = [
                x_ps.tile([VW, QCH], f32, tag="x", name=f"x_{it}_{hh}")
                for hh in range(2)
            ]
            p_tiles = []

            def emit_av(kt):
                for hh in range(2):
                    nc.tensor.matmul(
                        x_tiles[hh],
                        v_s[:, kt, 2 * p3 + hh, :],
                        p_tiles[kt][:, hh * QCH : (hh + 1) * QCH],
                        start=(kt == 0),
                        stop=(kt == NKT - 1),
                    )

            todo = list(drip[it])
            for kt in range(NKT):
                lt = lt_ps.tile([P, 2 * QCH], f32, tag="lt")
                for hh in range(2):
                    nc.tensor.matmul(
                        lt[:, hh * QCH : (hh + 1) * QCH],
                        kT_t[p3][rows[hh], kt * P : (kt + 1) * P],
                        qT_t[p3][rows[hh], qsl],
                        start=True,
                        stop=True,
                    )
                pt = pp.tile([P, 2 * QCH], bf16, tag="p")
                nc.scalar.activation(pt, lt, EXP)
                p_tiles.append(pt)
                if kt >= 2:
                    emit_av(kt - 2)
                if todo:
                    todo.pop(0)()
            emit_av(NKT - 2)
            emit_av(NKT - 1)
            while todo:
                todo.pop(0)()
            # normalization drain: 1/denom broadcast via DRAM round-trip
            for hh in range(2):
                r0 = small.tile([1, QCH], f32, tag="r0")
                nc.vector.tensor_copy(r0, x_tiles[hh][DH : DH + 1, :])
                r = small.tile([1, QCH], f32, tag="r")
                nc.vector.reciprocal_approx_fast(r, r0)
                rd = drp.tile([1, QCH], f32, tag="rd")
                nc.gpsimd.dma_start(out=rd, in_=r)
                rb = brd.tile([DH, QCH], f32, tag="rb")
                nc.gpsimd.dma_start(out=rb, in_=rd.to_broadcast([DH, QCH]))
                if hh == 0:
                    nc.vector.tensor_mul(
                        xn_t[p3][0:DH, qsl], x_tiles[0][0:DH, :], rb
                    )
                else:
                    tmp = brd.tile([DH, QCH], bf16, tag="tmp")
                    nc.vector.tensor_mul(tmp, x_tiles[1][0:DH, :], rb)
                    nc.sync.dma_start(out=xn_t[p3][DH : 2 * DH, qsl], in_=tmp)
        # final chunk's output projection
        for mt in range(C // P):
            oproj_chain(3, mt)

    nc.finalize()
    return nc


def _get_prog():
    global _prog
    if _prog is None:
        _prog = _build()
    return _prog


def _shard_inputs(query, key, value, Wq, Wk, Wv, Wo):
    from ml_dtypes import bfloat16

    in_maps = []
    for core in range(8):
        b, hg = core // 2, core % 2
        sl = slice(hg * HB, (hg + 1) * HB)
        in_maps.append(
            {
                "xq": np.ascontiguousarray(query[b].T).astype(bfloat16),
                "xk": np.ascontiguousarray(key[b].T).astype(bfloat16),
                "xv": np.ascontiguousarray(value[b].T).astype(bfloat16),
                "wq": np.ascontiguousarray((Wq[sl, :] * SCALE).T).astype(bfloat16),
                "wk": np.ascontiguousarray(Wk[sl, :].T).astype(bfloat16),
                "wv": np.ascontiguousarray(Wv[sl, :].T).astype(bfloat16),
                "wo": np.ascontiguousarray(Wo[:, sl].T).astype(bfloat16),
            }
        )
    return in_maps


def kernel(query, key, value, Wq, Wk, Wv, Wo, bo):
    query, key, value = np.asarray(query), np.asarray(key), np.asarray(value)
    Wq, Wk, Wv, Wo = np.asarray(Wq), np.asarray(Wk), np.asarray(Wv), np.asarray(Wo)
    bo = np.asarray(bo).astype(np.float32)

    from concourse.bass_utils import run_bass_kernel_spmd

    nc = _get_prog()
    in_maps = _shard_inputs(query, key, value, Wq, Wk, Wv, Wo)
    res = run_bass_kernel_spmd(nc, in_maps, list(range(8))).results

    out = np.empty((B, NQ, C), np.float32)
    for b in range(B):
        acc = res[2 * b]["out"].astype(np.float32) + res[2 * b + 1]["out"].astype(
            np.float32
        )
        out[b] = acc.T + bo[None, :]
    return out


# revision 25
# speedup vs baseline: 1.4529x; 1.0543x over previous
"""Cross-attention kernel for 8 Trainium2 NeuronCores.

Sharding: 8 cores = 4 batches x 2 head-groups (6 heads each).
Per core (b, hg), with all activations pre-transposed on host:
  qT = (Wq_hg*scale).T' @ xqT   [384, 2048]   (weights column-split)
  kT = Wk_hg' @ xkT             [384, 2048]
  v  = xvT' @ Wv_hg.T           [2048, 384]  (+ ones column per head)
  per head h: lt = k_h qT_h     [2048k, 2048q] (logits transposed, bf16 PSUM)
              p  = exp(lt)      (no max-subtraction: logits are O(1))
              [x; d] = [v_h|1].T @ p   -> x rows 0..63, denominators row 64
              xn = x * (1/d)    (partition-broadcast of 1/d via DRAM DMA)
  outT_partial = Wo_hg.T' @ xn  [768, 2048]  (DMA'd straight from PSUM)
Host: out[b] = (partial[2b] + partial[2b+1]).T + bo.

Pipeline: per (q-chunk, head-pair) iteration, QK pairs run row-tiled
concurrently (K=64 heads at partitions 0:64/64:128), exp consumes
[128,2048] bf16 PSUM tiles, AV trails exp by 2 kt-pairs, and the
projection chains are dripped into the PE stream to keep HAM warm.
"""

import sys

import numpy as np

for _p in ("/opt/trn_rl_repo",):
    if _p not in sys.path:
        sys.path.insert(0, _p)

B, NQ, NK, C = 4, 2048, 2048, 768
H, DH = 12, 64
HPC, HB = 6, 384  # heads per core, head-block width
P = 128
KT = C // P  # 6 contraction k-tiles for projections
QCH = 512  # query-chunk width
NCH = NQ // QCH  # 4 query chunks
NKT = NK // P  # 16 key tiles
SCALE = DH**-0.5  # folded into Wq on host (exactly 0.125)
VW = DH + 1  # v block width per head incl. ones column

_prog = None


def _build():
    from contextlib import ExitStack

    import concourse.bass as bass
    import concourse.tile as tile
    from concourse import mybir
    from concourse.bacc import Bacc

    f32 = mybir.dt.float32
    bf16 = mybir.dt.bfloat16
    EXP = mybir.ActivationFunctionType.Exp

    nc = Bacc()
    xq_d = nc.declare_dram_parameter("xq", [C, NQ], bf16, isOutput=False)
    xk_d = nc.declare_dram_parameter("xk", [C, NK], bf16, isOutput=False)
    xv_d = nc.declare_dram_parameter("xv", [C, NK], bf16, isOutput=False)
    wq_d = nc.declare_dram_parameter("wq", [C, HB], bf16, isOutput=False)
    wk_d = nc.declare_dram_parameter("wk", [C, HB], bf16, isOutput=False)
    wv_d = nc.declare_dram_parameter("wv", [C, HB], bf16, isOutput=False)
    wo_d = nc.declare_dram_parameter("wo", [HB, C], bf16, isOutput=False)
    out_d = nc.declare_dram_parameter("out", [C, NQ], bf16, isOutput=True)

    with tile.TileContext(nc) as tc, ExitStack() as ctx:
        const = ctx.enter_context(tc.tile_pool(name="const", bufs=1))
        xin = ctx.enter_context(tc.tile_pool(name="xin", bufs=KT))
        qk = ctx.enter_context(tc.tile_pool(name="qk", bufs=3))
        pp = ctx.enter_context(tc.tile_pool(name="pp", bufs=8))
        xnp = ctx.enter_context(tc.tile_pool(name="xnp", bufs=3))
        small = ctx.enter_context(tc.tile_pool(name="small", bufs=4))
        brd = ctx.enter_context(tc.tile_pool(name="brd", bufs=4))
        ost = ctx.enter_context(tc.tile_pool(name="ost", bufs=3))
        drp = ctx.enter_context(tc.tile_pool(name="drp", bufs=4, space="DRAM"))
        proj_ps = ctx.enter_context(tc.tile_pool(name="proj_ps", bufs=2, space="PSUM"))
        lt_ps = ctx.enter_context(tc.tile_pool(name="lt_ps", bufs=2, space="PSUM"))
        x_ps = ctx.enter_context(tc.tile_pool(name="x_ps", bufs=2, space="PSUM"))

        # ---- weights + inputs to SBUF
        wq_s = const.tile([P, KT, HB], bf16, tag="wq")
        wk_s = const.tile([P, KT, HB], bf16, tag="wk")
        wv_s = const.tile([P, KT, HB], bf16, tag="wv")
        wo_s = const.tile([P, HB // P, C], bf16, tag="wo")
        nc.sync.dma_start(out=wq_s, in_=wq_d.rearrange("(k p) m -> p k m", p=P))
        nc.sync.dma_start(out=wk_s, in_=wk_d.rearrange("(k p) m -> p k m", p=P))
        nc.sync.dma_start(out=wv_s, in_=wv_d.rearrange("(k p) m -> p k m", p=P))
        nc.sync.dma_start(out=wo_s, in_=wo_d.rearrange("(k p) m -> p k m", p=P))

        xq_t, xk_t, xv_t = [], [], []
        for name, dram, lst in (
            ("xk", xk_d, xk_t),
            ("xq", xq_d, xq_t),
            ("xv", xv_d, xv_t),
        ):
            for k in range(KT):
                t = xin.tile([P, NQ], bf16, tag=name, name=f"{name}_{k}")
                nc.sync.dma_start(
                    out=t, in_=dram.rearrange("(k p) m -> p k m", p=P)[:, k, :]
                )
                lst.append(t)

        # v with a ones column per head: [128, kt, head, 65]
        v_s = const.tile([P, NKT, HPC, VW], bf16, tag="v")
        nc.vector.memset(v_s[:, :, :, DH : DH + 1], 1.0)

        # Warm-up during the input-DMA window: ~10us of dummy matmuls gets
        # HAM to K=8/8 before real work, and a dummy exp absorbs the ~2.7us
        # ACT table load off the critical path.  Both depend only on the
        # (small, early) weight DMAs.
        warm_ps = lt_ps.tile([P, 2 * QCH], f32, tag="lt", name="warm")
        for i in range(28):
            nc.tensor.matmul(
                warm_ps[:, 0:HB],
                wq_s[:, i % KT, 0:P],
                wq_s[:, (i + 1) % KT, :],
                start=True,
                stop=True,
            )
        warm_act = small.tile([1, QCH], f32, tag="r0", name="warm_act")
        nc.scalar.activation(warm_act[:, 0:HB], warm_ps[0:1, 0:HB], EXP)

        qT_t = [qk.tile([P, NQ], bf16, tag="qT", name=f"qT{i}") for i in range(3)]
        kT_t = [qk.tile([P, NK], bf16, tag="kT", name=f"kT{i}") for i in range(3)]
        xn_t = [xnp.tile([P, NQ], bf16, tag="xn", name=f"xn{i}") for i in range(3)]

        # ---- deferred projection-chain emitters -------------------------
        def qk_chain(w_s, src, dst, mt, j4):
            ps = proj_ps.tile([P, QCH], f32, tag="proj")
            for k in range(KT):
                nc.tensor.matmul(
                    ps,
                    w_s[:, k, mt * P : (mt + 1) * P],
                    src[k][:, j4 * QCH : (j4 + 1) * QCH],
                    start=(k == 0),
                    stop=(k == KT - 1),
                )
            nc.vector.tensor_copy(dst[:, j4 * QCH : (j4 + 1) * QCH], ps)

        def v_chain(kt):
            ps = proj_ps.tile([P, QCH], f32, tag="proj")
            for k in range(KT):
                nc.tensor.matmul(
                    ps[:, 0:HB],
                    xv_t[k][:, kt * P : (kt + 1) * P],
                    wv_s[:, k, :],
                    start=(k == 0),
                    stop=(k == KT - 1),
                )
            nc.vector.tensor_copy(
                v_s[:, kt, :, 0:DH], ps[:, 0:HB].rearrange("p (h m) -> p h m", m=DH)
            )

        def oproj_chain(j4, mt):
            qsl = slice(j4 * QCH, (j4 + 1) * QCH)
            ps = proj_ps.tile([P, QCH], f32, tag="proj")
            for k3 in range(HB // P):
                nc.tensor.matmul(
                    ps,
                    wo_s[:, k3, mt * P : (mt + 1) * P],
                    xn_t[k3][:, qsl],
                    start=(k3 == 0),
                    stop=(k3 == HB // P - 1),
                )
            o = ost.tile([P, QCH], bf16, tag="o")
            nc.vector.tensor_copy(o, ps)
            nc.sync.dma_start(out=out_d[mt * P : (mt + 1) * P, qsl], in_=o)

        # drip schedule: iteration index -> list of thunks to interleave.
        # Iteration order is pair-outer: it = p3*4 + j4.  kT_t[p3] must be
        # FULLY projected (all key chunks) before iteration p3*4; qT_t[p3]
        # only needs q-chunk j4 by iteration p3*4+j4; v chains before the
        # AVs that use them; out-proj(j4) after iteration 8+j4.
        def q_chain(p3, j4):
            qk_chain(wq_s, xq_t, qT_t[p3], p3, j4)

        def k_chain(p3, kc):
            qk_chain(wk_s, xk_t, kT_t[p3], p3, kc)

        drip = {i: [] for i in range(12)}
        upfront = (
            [lambda kc=kc: k_chain(0, kc) for kc in range(4)]
            + [lambda: q_chain(0, 0)]
            + [lambda kt=kt: v_chain(kt) for kt in range(4)]
        )
        drip[0] = [lambda kt=kt: v_chain(kt) for kt in range(4, 16)] + [
            lambda: q_chain(0, 1)
        ]
        drip[1] = [
            lambda: q_chain(0, 2),
            lambda: k_chain(1, 0),
            lambda: k_chain(1, 1),
        ]
        drip[2] = [
            lambda: q_chain(0, 3),
            lambda: k_chain(1, 2),
            lambda: k_chain(1, 3),
        ]
        drip[3] = [lambda: q_chain(1, 0), lambda: k_chain(2, 0)]
        drip[4] = [lambda: q_chain(1, 1), lambda: k_chain(2, 1)]
        drip[5] = [lambda: q_chain(1, 2), lambda: k_chain(2, 2)]
        drip[6] = [lambda: q_chain(1, 3), lambda: k_chain(2, 3)]
        drip[7] = [lambda: q_chain(2, 0)]
        drip[8] = [lambda: q_chain(2, 1)]
        drip[9] = [lambda: q_chain(2, 2)]
        drip[10] = [lambda: q_chain(2, 3)] + [
            lambda mt=mt: oproj_chain(0, mt) for mt in range(C // P)
        ]
        drip[11] = [lambda mt=mt: oproj_chain(1, mt) for mt in range(C // P)] + [
            lambda mt=mt: oproj_chain(2, mt) for mt in range(C // P)
        ]
        for th in upfront:
            th()

        # ---- attention pipeline: flat cross-iteration software pipeline.
        # Slots are (it, kt); QK+exp lead, AV trails AV_LAG slots behind
        # (crossing iteration boundaries), drains are emitted as soon as an
        # iteration's last AV is emitted — mid-next-iteration — so the
        # 2-deep x_ps ring never stalls the PE.
        from collections import deque

        AV_LAG = 4
        rows = (slice(0, DH), slice(DH, 2 * DH))
        x_tiles = {}
        p_tiles = {}
        av_q = deque()

        def emit_av(it, kt):
            p3 = it // 4
            pt = p_tiles.pop((it, kt))
            for hh in range(2):
                nc.tensor.matmul(
                    x_tiles[it][hh],
                    v_s[:, kt, 2 * p3 + hh, :],
                    pt[:, hh * QCH : (hh + 1) * QCH],
                    start=(kt == 0),
                    stop=(kt == NKT - 1),
                )

        def emit_drain(it):
            p3, j4 = divmod(it, 4)
            qsl = slice(j4 * QCH, (j4 + 1) * QCH)
            for hh in range(2):
                r0 = small.tile([1, QCH], f32, tag="r0")
                nc.vector.tensor_copy(r0, x_tiles[it][hh][DH : DH + 1, :])
                r = small.tile([1, QCH], f32, tag="r")
                nc.vector.reciprocal_approx_fast(r, r0)
                rd = drp.tile([1, QCH], f32, tag="rd")
                nc.gpsimd.dma_start(out=rd, in_=r)
                rb = brd.tile([DH, QCH], f32, tag="rb")
                nc.gpsimd.dma_start(out=rb, in_=rd.to_broadcast([DH, QCH]))
                if hh == 0:
                    nc.vector.tensor_mul(
                        xn_t[p3][0:DH, qsl], x_tiles[it][0][0:DH, :], rb
                    )
                else:
                    tmp = brd.tile([DH, QCH], bf16, tag="tmp")
                    nc.vector.tensor_mul(tmp, x_tiles[it][1][0:DH, :], rb)
                    nc.sync.dma_start(out=xn_t[p3][DH : 2 * DH, qsl], in_=tmp)
            del x_tiles[it]

        todo = []
        for it in range(12):
            p3, j4 = divmod(it, 4)
            qsl = slice(j4 * QCH, (j4 + 1) * QCH)
            x_tiles[it] = [
                x_ps.tile([VW, QCH], f32, tag="x", name=f"x_{it}_{hh}")
                for hh in range(2)
            ]
            todo.extend(drip[it])
            for kt in range(NKT):
                # trailing AV first: its deps are oldest
                if len(av_q) >= AV_LAG:
                    a_it, a_kt = av_q.popleft()
                    emit_av(a_it, a_kt)
                    if a_kt == NKT - 1:
                        emit_drain(a_it)
                lt = lt_ps.tile([P, 2 * QCH], f32, tag="lt")
                for hh in range(2):
                    nc.tensor.matmul(
                        lt[:, hh * QCH : (hh + 1) * QCH],
                        kT_t[p3][rows[hh], kt * P : (kt + 1) * P],
                        qT_t[p3][rows[hh], qsl],
                        start=True,
                        stop=True,
                    )
                pt = pp.tile([P, 2 * QCH], bf16, tag="p")
                nc.scalar.activation(pt, lt, EXP)
                p_tiles[(it, kt)] = pt
                av_q.append((it, kt))
                if todo:
                    todo.pop(0)()
        while av_q:
            a_it, a_kt = av_q.popleft()
            emit_av(a_it, a_kt)
            if a_kt == NKT - 1:
                emit_drain(a_it)
        while todo:
            todo.pop(0)()
        # final chunk's output projection
        for mt in range(C // P):
            oproj_chain(3, mt)

    nc.finalize()
    return nc


def _get_prog():
    global _prog
    if _prog is None:
        _prog = _build()
    return _prog


def _shard_inputs(query, key, value, Wq, Wk, Wv, Wo):
    from ml_dtypes import bfloat16

    in_maps = []
    for core in range(8):
        b, hg = core // 2, core % 2
        sl = slice(hg * HB, (hg + 1) * HB)
        in_maps.append(
            {
                "xq": np.ascontiguousarray(query[b].T).astype(bfloat16),
                "xk": np.ascontiguousarray(key[b].T).astype(bfloat16),
                "xv": np.ascontiguousarray(value[b].T).astype(bfloat16),
                "wq": np.ascontiguousarray((Wq[sl, :] * SCALE).T).astype(bfloat16),
                "wk": np.ascontiguousarray(Wk[sl, :].T).astype(bfloat16),
                "wv": np.ascontiguousarray(Wv[sl, :].T).astype(bfloat16),
                "wo": np.ascontiguousarray(Wo[:, sl].T).astype(bfloat16),
            }
        )
    return in_maps


def kernel(query, key, value, Wq, Wk, Wv, Wo, bo):
    query, key, value = np.asarray(query), np.asarray(key), np.asarray(value)
    Wq, Wk, Wv, Wo = np.asarray(Wq), np.asarray(Wk), np.asarray(Wv), np.asarray(Wo)
    bo = np.asarray(bo).astype(np.float32)

    from concourse.bass_utils import run_bass_kernel_spmd

    nc = _get_prog()
    in_maps = _shard_inputs(query, key, value, Wq, Wk, Wv, Wo)
    res = run_bass_kernel_spmd(nc, in_maps, list(range(8))).results

    out = np.empty((B, NQ, C), np.float32)
    for b in range(B):
        acc = res[2 * b]["out"].astype(np.float32) + res[2 * b + 1]["out"].astype(
            np.float32
        )
        out[b] = acc.T + bo[None, :]
    return out


# revision 26
# speedup vs baseline: 1.6763x; 1.1538x over previous
"""Cross-attention kernel for 8 Trainium2 NeuronCores.

Sharding: 8 cores = 4 batches x 2 head-groups (6 heads each).
Per core (b, hg), with all activations pre-transposed on host:
  qT = (Wq_hg*scale).T' @ xqT   [384, 2048]   (weights column-split)
  kT = Wk_hg' @ xkT             [384, 2048]
  v  = xvT' @ Wv_hg.T           [2048, 384]  (+ ones column per head)
  per head h: lt = k_h qT_h     [2048k, 2048q] (logits transposed, bf16 PSUM)
              p  = exp(lt)      (no max-subtraction: logits are O(1))
              [x; d] = [v_h|1].T @ p   -> x rows 0..63, denominators row 64
              xn = x * (1/d)    (partition-broadcast of 1/d via DRAM DMA)
  outT_partial = Wo_hg.T' @ xn  [768, 2048]  (DMA'd straight from PSUM)
Host: out[b] = (partial[2b] + partial[2b+1]).T + bo.

Pipeline: per (q-chunk, head-pair) iteration, QK pairs run row-tiled
concurrently (K=64 heads at partitions 0:64/64:128), exp consumes
[128,2048] bf16 PSUM tiles, AV trails exp by 2 kt-pairs, and the
projection chains are dripped into the PE stream to keep HAM warm.
"""

import sys

import numpy as np

for _p in ("/opt/trn_rl_repo",):
    if _p not in sys.path:
        sys.path.insert(0, _p)

B, NQ, NK, C = 4, 2048, 2048, 768
H, DH = 12, 64
HPC, HB = 6, 384  # heads per core, head-block width
P = 128
KT = C // P  # 6 contraction k-tiles for projections
QCH = 512  # query-chunk width
NCH = NQ // QCH  # 4 query chunks
NKT = NK // P  # 16 key tiles
SCALE = DH**-0.5  # folded into Wq on host (exactly 0.125)
VW = DH + 1  # v block width per head incl. ones column

_prog = None


def _build():
    from contextlib import ExitStack

    import concourse.bass as bass
    import concourse.tile as tile
    from concourse import mybir
    from concourse.bacc import Bacc

    f32 = mybir.dt.float32
    bf16 = mybir.dt.bfloat16
    EXP = mybir.ActivationFunctionType.Exp

    nc = Bacc()
    xq_d = nc.declare_dram_parameter("xq", [C, NQ], bf16, isOutput=False)
    xk_d = nc.declare_dram_parameter("xk", [C, NK], bf16, isOutput=False)
    xv_d = nc.declare_dram_parameter("xv", [C, NK], bf16, isOutput=False)
    wq_d = nc.declare_dram_parameter("wq", [C, HB], bf16, isOutput=False)
    wk_d = nc.declare_dram_parameter("wk", [C, HB], bf16, isOutput=False)
    wv_d = nc.declare_dram_parameter("wv", [C, HB], bf16, isOutput=False)
    wo_d = nc.declare_dram_parameter("wo", [HB, C], bf16, isOutput=False)
    out_d = nc.declare_dram_parameter("out", [C, NQ], bf16, isOutput=True)

    with tile.TileContext(nc) as tc, ExitStack() as ctx:
        const = ctx.enter_context(tc.tile_pool(name="const", bufs=1))
        xin = ctx.enter_context(tc.tile_pool(name="xin", bufs=KT))
        qk = ctx.enter_context(tc.tile_pool(name="qk", bufs=3))
        pp = ctx.enter_context(tc.tile_pool(name="pp", bufs=12))
        xnp = ctx.enter_context(tc.tile_pool(name="xnp", bufs=3))
        small = ctx.enter_context(tc.tile_pool(name="small", bufs=3))
        brd = ctx.enter_context(tc.tile_pool(name="brd", bufs=4))
        ost = ctx.enter_context(tc.tile_pool(name="ost", bufs=3))
        drp = ctx.enter_context(tc.tile_pool(name="drp", bufs=4, space="DRAM"))
        proj_ps = ctx.enter_context(tc.tile_pool(name="proj_ps", bufs=2, space="PSUM"))
        lt_ps = ctx.enter_context(tc.tile_pool(name="lt_ps", bufs=2, space="PSUM"))
        x_ps = ctx.enter_context(tc.tile_pool(name="x_ps", bufs=2, space="PSUM"))

        # ---- weights + inputs to SBUF
        wq_s = const.tile([P, KT, HB], bf16, tag="wq")
        wk_s = const.tile([P, KT, HB], bf16, tag="wk")
        wv_s = const.tile([P, KT, HB], bf16, tag="wv")
        wo_s = const.tile([P, HB // P, C], bf16, tag="wo")
        nc.sync.dma_start(out=wq_s, in_=wq_d.rearrange("(k p) m -> p k m", p=P))
        nc.sync.dma_start(out=wk_s, in_=wk_d.rearrange("(k p) m -> p k m", p=P))
        nc.sync.dma_start(out=wv_s, in_=wv_d.rearrange("(k p) m -> p k m", p=P))
        nc.sync.dma_start(out=wo_s, in_=wo_d.rearrange("(k p) m -> p k m", p=P))

        xq_t, xk_t, xv_t = [], [], []
        for name, dram, lst in (
            ("xk", xk_d, xk_t),
            ("xq", xq_d, xq_t),
            ("xv", xv_d, xv_t),
        ):
            for k in range(KT):
                t = xin.tile([P, NQ], bf16, tag=name, name=f"{name}_{k}")
                nc.sync.dma_start(
                    out=t, in_=dram.rearrange("(k p) m -> p k m", p=P)[:, k, :]
                )
                lst.append(t)

        # v with a ones column per head: [128, kt, head, 65]
        v_s = const.tile([P, NKT, HPC, VW], bf16, tag="v")
        nc.vector.memset(v_s[:, :, :, DH : DH + 1], 1.0)

        # Warm-up during the input-DMA window: ~10us of dummy matmuls gets
        # HAM to K=8/8 before real work, and a dummy exp absorbs the ~2.7us
        # ACT table load off the critical path.  Both depend only on the
        # (small, early) weight DMAs.
        warm_ps = lt_ps.tile([P, 2 * QCH], f32, tag="lt", name="warm")
        for i in range(28):
            nc.tensor.matmul(
                warm_ps[:, 0:HB],
                wq_s[:, i % KT, 0:P],
                wq_s[:, (i + 1) % KT, :],
                start=True,
                stop=True,
            )
        warm_act = small.tile([1, QCH], f32, tag="r0", name="warm_act")
        nc.scalar.activation(warm_act[:, 0:HB], warm_ps[0:1, 0:HB], EXP)

        qT_t = [qk.tile([P, NQ], bf16, tag="qT", name=f"qT{i}") for i in range(3)]
        kT_t = [qk.tile([P, NK], bf16, tag="kT", name=f"kT{i}") for i in range(3)]
        xn_t = [xnp.tile([P, NQ], bf16, tag="xn", name=f"xn{i}") for i in range(3)]

        # ---- deferred projection-chain emitters -------------------------
        def qk_chain(w_s, src, dst, mt, j4):
            ps = proj_ps.tile([P, QCH], f32, tag="proj")
            for k in range(KT):
                nc.tensor.matmul(
                    ps,
                    w_s[:, k, mt * P : (mt + 1) * P],
                    src[k][:, j4 * QCH : (j4 + 1) * QCH],
                    start=(k == 0),
                    stop=(k == KT - 1),
                )
            nc.vector.tensor_copy(dst[:, j4 * QCH : (j4 + 1) * QCH], ps)

        def v_chain(kt):
            ps = proj_ps.tile([P, QCH], f32, tag="proj")
            for k in range(KT):
                nc.tensor.matmul(
                    ps[:, 0:HB],
                    xv_t[k][:, kt * P : (kt + 1) * P],
                    wv_s[:, k, :],
                    start=(k == 0),
                    stop=(k == KT - 1),
                )
            nc.vector.tensor_copy(
                v_s[:, kt, :, 0:DH], ps[:, 0:HB].rearrange("p (h m) -> p h m", m=DH)
            )

        def oproj_chain(j4, mt):
            qsl = slice(j4 * QCH, (j4 + 1) * QCH)
            ps = proj_ps.tile([P, QCH], f32, tag="proj")
            for k3 in range(HB // P):
                nc.tensor.matmul(
                    ps,
                    wo_s[:, k3, mt * P : (mt + 1) * P],
                    xn_t[k3][:, qsl],
                    start=(k3 == 0),
                    stop=(k3 == HB // P - 1),
                )
            o = ost.tile([P, QCH], bf16, tag="o")
            nc.vector.tensor_copy(o, ps)
            nc.sync.dma_start(out=out_d[mt * P : (mt + 1) * P, qsl], in_=o)

        # drip schedule: iteration index -> list of thunks to interleave.
        # Iteration order is pair-outer: it = p3*4 + j4.  kT_t[p3] must be
        # FULLY projected (all key chunks) before iteration p3*4; qT_t[p3]
        # only needs q-chunk j4 by iteration p3*4+j4; v chains before the
        # AVs that use them; out-proj(j4) after iteration 8+j4.
        def q_chain(p3, j4):
            qk_chain(wq_s, xq_t, qT_t[p3], p3, j4)

        def k_chain(p3, kc):
            qk_chain(wk_s, xk_t, kT_t[p3], p3, kc)

        drip = {i: [] for i in range(12)}
        upfront = (
            [lambda kc=kc: k_chain(0, kc) for kc in range(4)]
            + [lambda: q_chain(0, 0)]
            + [lambda kt=kt: v_chain(kt) for kt in range(4)]
        )
        drip[0] = [lambda kt=kt: v_chain(kt) for kt in range(4, 16)] + [
            lambda: q_chain(0, 1)
        ]
        drip[1] = [
            lambda: q_chain(0, 2),
            lambda: k_chain(1, 0),
            lambda: k_chain(1, 1),
        ]
        drip[2] = [
            lambda: q_chain(0, 3),
            lambda: k_chain(1, 2),
            lambda: k_chain(1, 3),
        ]
        drip[3] = [lambda: q_chain(1, 0), lambda: k_chain(2, 0)]
        drip[4] = [lambda: q_chain(1, 1), lambda: k_chain(2, 1)]
        drip[5] = [lambda: q_chain(1, 2), lambda: k_chain(2, 2)]
        drip[6] = [lambda: q_chain(1, 3), lambda: k_chain(2, 3)]
        drip[7] = [lambda: q_chain(2, 0)]
        drip[8] = [lambda: q_chain(2, 1)]
        drip[9] = [lambda: q_chain(2, 2)]
        drip[10] = [lambda: q_chain(2, 3)] + [
            lambda mt=mt: oproj_chain(0, mt) for mt in range(C // P)
        ]
        drip[11] = [lambda mt=mt: oproj_chain(1, mt) for mt in range(C // P)] + [
            lambda mt=mt: oproj_chain(2, mt) for mt in range(C // P)
        ]
        for th in upfront:
            th()

        # ---- attention pipeline: flat cross-iteration software pipeline.
        # Slots are (it, kt); QK+exp lead, AV trails AV_LAG slots behind
        # (crossing iteration boundaries), drains are emitted as soon as an
        # iteration's last AV is emitted — mid-next-iteration — so the
        # 2-deep x_ps ring never stalls the PE.
        from collections import deque

        AV_LAG = 4
        rows = (slice(0, DH), slice(DH, 2 * DH))
        x_tiles = {}
        p_tiles = {}
        av_q = deque()

        def emit_av(it, kt):
            p3 = it // 4
            pt = p_tiles.pop((it, kt))
            for hh in range(2):
                nc.tensor.matmul(
                    x_tiles[it][hh],
                    v_s[:, kt, 2 * p3 + hh, :],
                    pt[:, hh * QCH : (hh + 1) * QCH],
                    start=(kt == 0),
                    stop=(kt == NKT - 1),
                )

        def emit_drain(it):
            p3, j4 = divmod(it, 4)
            qsl = slice(j4 * QCH, (j4 + 1) * QCH)
            for hh in range(2):
                r0 = small.tile([1, QCH], f32, tag="r0")
                nc.vector.tensor_copy(r0, x_tiles[it][hh][DH : DH + 1, :])
                r = small.tile([1, QCH], f32, tag="r")
                nc.vector.reciprocal_approx_fast(r, r0)
                rd = drp.tile([1, QCH], f32, tag="rd")
                nc.gpsimd.dma_start(out=rd, in_=r)
                rb = brd.tile([DH, QCH], f32, tag="rb")
                nc.gpsimd.dma_start(out=rb, in_=rd.to_broadcast([DH, QCH]))
                if hh == 0:
                    nc.vector.tensor_mul(
                        xn_t[p3][0:DH, qsl], x_tiles[it][0][0:DH, :], rb
                    )
                else:
                    tmp = brd.tile([DH, QCH], bf16, tag="tmp")
                    nc.vector.tensor_mul(tmp, x_tiles[it][1][0:DH, :], rb)
                    nc.sync.dma_start(out=xn_t[p3][DH : 2 * DH, qsl], in_=tmp)
            del x_tiles[it]

        todo = []
        for it in range(12):
            p3, j4 = divmod(it, 4)
            qsl = slice(j4 * QCH, (j4 + 1) * QCH)
            x_tiles[it] = [
                x_ps.tile([VW, QCH], f32, tag="x", name=f"x_{it}_{hh}")
                for hh in range(2)
            ]
            todo.extend(drip[it])
            for kt in range(NKT):
                # trailing AV first: its deps are oldest
                if len(av_q) >= AV_LAG:
                    a_it, a_kt = av_q.popleft()
                    emit_av(a_it, a_kt)
                    if a_kt == NKT - 1:
                        emit_drain(a_it)
                lt = lt_ps.tile([P, 2 * QCH], f32, tag="lt")
                for hh in range(2):
                    nc.tensor.matmul(
                        lt[:, hh * QCH : (hh + 1) * QCH],
                        kT_t[p3][rows[hh], kt * P : (kt + 1) * P],
                        qT_t[p3][rows[hh], qsl],
                        start=True,
                        stop=True,
                    )
                pt = pp.tile([P, 2 * QCH], bf16, tag="p")
                nc.scalar.activation(pt, lt, EXP)
                p_tiles[(it, kt)] = pt
                av_q.append((it, kt))
                if todo:
                    todo.pop(0)()
        while av_q:
            a_it, a_kt = av_q.popleft()
            emit_av(a_it, a_kt)
            if a_kt == NKT - 1:
                emit_drain(a_it)
        while todo:
            todo.pop(0)()
        # final chunk's output projection
        for mt in range(C // P):
            oproj_chain(3, mt)

    nc.finalize()
    return nc


def _get_prog():
    global _prog
    if _prog is None:
        _prog = _build()
    return _prog


def _shard_inputs(query, key, value, Wq, Wk, Wv, Wo):
    from ml_dtypes import bfloat16

    in_maps = []
    for core in range(8):
        b, hg = core // 2, core % 2
        sl = slice(hg * HB, (hg + 1) * HB)
        in_maps.append(
            {
                "xq": np.ascontiguousarray(query[b].T).astype(bfloat16),
                "xk": np.ascontiguousarray(key[b].T).astype(bfloat16),
                "xv": np.ascontiguousarray(value[b].T).astype(bfloat16),
                "wq": np.ascontiguousarray((Wq[sl, :] * SCALE).T).astype(bfloat16),
                "wk": np.ascontiguousarray(Wk[sl, :].T).astype(bfloat16),
                "wv": np.ascontiguousarray(Wv[sl, :].T).astype(bfloat16),
                "wo": np.ascontiguousarray(Wo[:, sl].T).astype(bfloat16),
            }
        )
    return in_maps


def kernel(query, key, value, Wq, Wk, Wv, Wo, bo):
    query, key, value = np.asarray(query), np.asarray(key), np.asarray(value)
    Wq, Wk, Wv, Wo = np.asarray(Wq), np.asarray(Wk), np.asarray(Wv), np.asarray(Wo)
    bo = np.asarray(bo).astype(np.float32)

    from concourse.bass_utils import run_bass_kernel_spmd

    nc = _get_prog()
    in_maps = _shard_inputs(query, key, value, Wq, Wk, Wv, Wo)
    res = run_bass_kernel_spmd(nc, in_maps, list(range(8))).results

    out = np.empty((B, NQ, C), np.float32)
    for b in range(B):
        acc = res[2 * b]["out"].astype(np.float32) + res[2 * b + 1]["out"].astype(
            np.float32
        )
        out[b] = acc.T + bo[None, :]
    return out


# revision 27
# speedup vs baseline: 1.8872x; 1.1258x over previous
"""Cross-attention kernel for 8 Trainium2 NeuronCores.

Sharding: 8 cores = 4 batches x 2 head-groups (6 heads each).
Per core (b, hg), with all activations pre-transposed on host:
  qT = (Wq_hg*scale).T' @ xqT   [384, 2048]   (weights column-split)
  kT = Wk_hg' @ xkT             [384, 2048]
  v  = xvT' @ Wv_hg.T           [2048, 384]  (+ ones column per head)
  per head h: lt = k_h qT_h     [2048k, 2048q] (logits transposed, bf16 PSUM)
              p  = exp(lt)      (no max-subtraction: logits are O(1))
              [x; d] = [v_h|1].T @ p   -> x rows 0..63, denominators row 64
              xn = x * (1/d)    (partition-broadcast of 1/d via DRAM DMA)
  outT_partial = Wo_hg.T' @ xn  [768, 2048]  (DMA'd straight from PSUM)
Host: out[b] = (partial[2b] + partial[2b+1]).T + bo.

Pipeline: per (q-chunk, head-pair) iteration, QK pairs run row-tiled
concurrently (K=64 heads at partitions 0:64/64:128), exp consumes
[128,2048] bf16 PSUM tiles, AV trails exp by 2 kt-pairs, and the
projection chains are dripped into the PE stream to keep HAM warm.
"""

import sys

import numpy as np

for _p in ("/opt/trn_rl_repo",):
    if _p not in sys.path:
        sys.path.insert(0, _p)

B, NQ, NK, C = 4, 2048, 2048, 768
H, DH = 12, 64
HPC, HB = 6, 384  # heads per core, head-block width
P = 128
KT = C // P  # 6 contraction k-tiles for projections
QCH = 512  # query-chunk width
NCH = NQ // QCH  # 4 query chunks
NKT = NK // P  # 16 key tiles
SCALE = DH**-0.5  # folded into Wq on host (exactly 0.125)
VW = DH + 1  # v block width per head incl. ones column

_prog = None


def _build():
    from contextlib import ExitStack

    import concourse.bass as bass
    import concourse.tile as tile
    from concourse import mybir
    from concourse.bacc import Bacc

    f32 = mybir.dt.float32
    bf16 = mybir.dt.bfloat16
    EXP = mybir.ActivationFunctionType.Exp

    nc = Bacc()
    xq_d = nc.declare_dram_parameter("xq", [C, NQ], bf16, isOutput=False)
    xk_d = nc.declare_dram_parameter("xk", [C, NK], bf16, isOutput=False)
    xv_d = nc.declare_dram_parameter("xv", [C, NK], bf16, isOutput=False)
    wq_d = nc.declare_dram_parameter("wq", [C, HB], bf16, isOutput=False)
    wk_d = nc.declare_dram_parameter("wk", [C, HB], bf16, isOutput=False)
    wv_d = nc.declare_dram_parameter("wv", [C, HB], bf16, isOutput=False)
    wo_d = nc.declare_dram_parameter("wo", [HB, C], bf16, isOutput=False)
    out_d = nc.declare_dram_parameter("out", [C, NQ], bf16, isOutput=True)

    with tile.TileContext(nc) as tc, ExitStack() as ctx:
        const = ctx.enter_context(tc.tile_pool(name="const", bufs=1))
        xin = ctx.enter_context(tc.tile_pool(name="xin", bufs=KT))
        qk = ctx.enter_context(tc.tile_pool(name="qk", bufs=3))
        pp = ctx.enter_context(tc.tile_pool(name="pp", bufs=12))
        xnp = ctx.enter_context(tc.tile_pool(name="xnp", bufs=3))
        small = ctx.enter_context(tc.tile_pool(name="small", bufs=3))
        brd = ctx.enter_context(tc.tile_pool(name="brd", bufs=4))
        ost = ctx.enter_context(tc.tile_pool(name="ost", bufs=3))
        xsb = ctx.enter_context(tc.tile_pool(name="xsb", bufs=3))
        drp = ctx.enter_context(tc.tile_pool(name="drp", bufs=4, space="DRAM"))
        proj_ps = ctx.enter_context(tc.tile_pool(name="proj_ps", bufs=2, space="PSUM"))
        lt_ps = ctx.enter_context(tc.tile_pool(name="lt_ps", bufs=2, space="PSUM"))
        x_ps = ctx.enter_context(tc.tile_pool(name="x_ps", bufs=2, space="PSUM"))

        # ---- weights + inputs to SBUF
        wq_s = const.tile([P, KT, HB], bf16, tag="wq")
        wk_s = const.tile([P, KT, HB], bf16, tag="wk")
        wv_s = const.tile([P, KT, HB], bf16, tag="wv")
        wo_s = const.tile([P, HB // P, C], bf16, tag="wo")
        nc.sync.dma_start(out=wq_s, in_=wq_d.rearrange("(k p) m -> p k m", p=P))
        nc.sync.dma_start(out=wk_s, in_=wk_d.rearrange("(k p) m -> p k m", p=P))
        nc.sync.dma_start(out=wv_s, in_=wv_d.rearrange("(k p) m -> p k m", p=P))
        nc.sync.dma_start(out=wo_s, in_=wo_d.rearrange("(k p) m -> p k m", p=P))

        xq_t, xk_t, xv_t = [], [], []
        for name, dram, lst in (
            ("xk", xk_d, xk_t),
            ("xq", xq_d, xq_t),
            ("xv", xv_d, xv_t),
        ):
            for k in range(KT):
                t = xin.tile([P, NQ], bf16, tag=name, name=f"{name}_{k}")
                nc.sync.dma_start(
                    out=t, in_=dram.rearrange("(k p) m -> p k m", p=P)[:, k, :]
                )
                lst.append(t)

        # v with a ones column per head: [128, kt, head, 65]
        v_s = const.tile([P, NKT, HPC, VW], bf16, tag="v")
        nc.vector.memset(v_s[:, :, :, DH : DH + 1], 1.0)

        # Warm-up during the input-DMA window: ~10us of dummy matmuls gets
        # HAM to K=8/8 before real work, and a dummy exp absorbs the ~2.7us
        # ACT table load off the critical path.  Both depend only on the
        # (small, early) weight DMAs.
        warm_ps = lt_ps.tile([P, 2 * QCH], f32, tag="lt", name="warm")
        for i in range(52):
            nc.tensor.matmul(
                warm_ps[:, 0:HB],
                wq_s[:, i % KT, 0:P],
                wq_s[:, (i + 1) % KT, :],
                start=True,
                stop=True,
            )
        warm_act = small.tile([1, QCH], f32, tag="r0", name="warm_act")
        nc.scalar.activation(warm_act[:, 0:HB], warm_ps[0:1, 0:HB], EXP)

        qT_t = [qk.tile([P, NQ], bf16, tag="qT", name=f"qT{i}") for i in range(3)]
        kT_t = [qk.tile([P, NK], bf16, tag="kT", name=f"kT{i}") for i in range(3)]
        xn_t = [xnp.tile([P, NQ], bf16, tag="xn", name=f"xn{i}") for i in range(3)]

        # ---- deferred projection-chain emitters -------------------------
        def qk_chain(w_s, src, dst, mt, j4):
            ps = proj_ps.tile([P, QCH], f32, tag="proj")
            for k in range(KT):
                nc.tensor.matmul(
                    ps,
                    w_s[:, k, mt * P : (mt + 1) * P],
                    src[k][:, j4 * QCH : (j4 + 1) * QCH],
                    start=(k == 0),
                    stop=(k == KT - 1),
                )
            nc.vector.tensor_copy(dst[:, j4 * QCH : (j4 + 1) * QCH], ps)

        def v_chain(kt):
            ps = proj_ps.tile([P, QCH], f32, tag="proj")
            for k in range(KT):
                nc.tensor.matmul(
                    ps[:, 0:HB],
                    xv_t[k][:, kt * P : (kt + 1) * P],
                    wv_s[:, k, :],
                    start=(k == 0),
                    stop=(k == KT - 1),
                )
            nc.vector.tensor_copy(
                v_s[:, kt, :, 0:DH], ps[:, 0:HB].rearrange("p (h m) -> p h m", m=DH)
            )

        def oproj_chain(j4, mt):
            qsl = slice(j4 * QCH, (j4 + 1) * QCH)
            ps = proj_ps.tile([P, QCH], f32, tag="proj")
            for k3 in range(HB // P):
                nc.tensor.matmul(
                    ps,
                    wo_s[:, k3, mt * P : (mt + 1) * P],
                    xn_t[k3][:, qsl],
                    start=(k3 == 0),
                    stop=(k3 == HB // P - 1),
                )
            o = ost.tile([P, QCH], bf16, tag="o")
            nc.vector.tensor_copy(o, ps)
            nc.sync.dma_start(out=out_d[mt * P : (mt + 1) * P, qsl], in_=o)

        # drip schedule: iteration index -> list of thunks to interleave.
        # Iteration order is pair-outer: it = p3*4 + j4.  kT_t[p3] must be
        # FULLY projected (all key chunks) before iteration p3*4; qT_t[p3]
        # only needs q-chunk j4 by iteration p3*4+j4; v chains before the
        # AVs that use them; out-proj(j4) after iteration 8+j4.
        def q_chain(p3, j4):
            qk_chain(wq_s, xq_t, qT_t[p3], p3, j4)

        def k_chain(p3, kc):
            qk_chain(wk_s, xk_t, kT_t[p3], p3, kc)

        drip = {i: [] for i in range(12)}
        upfront = (
            [lambda kc=kc: k_chain(0, kc) for kc in range(4)]
            + [lambda: q_chain(0, 0)]
            + [lambda kt=kt: v_chain(kt) for kt in range(4)]
        )
        drip[0] = [lambda kt=kt: v_chain(kt) for kt in range(4, 16)] + [
            lambda: q_chain(0, 1)
        ]
        drip[1] = [
            lambda: q_chain(0, 2),
            lambda: k_chain(1, 0),
            lambda: k_chain(1, 1),
        ]
        drip[2] = [
            lambda: q_chain(0, 3),
            lambda: k_chain(1, 2),
            lambda: k_chain(1, 3),
        ]
        drip[3] = [lambda: q_chain(1, 0), lambda: k_chain(2, 0)]
        drip[4] = [lambda: q_chain(1, 1), lambda: k_chain(2, 1)]
        drip[5] = [lambda: q_chain(1, 2), lambda: k_chain(2, 2)]
        drip[6] = [lambda: q_chain(1, 3), lambda: k_chain(2, 3)]
        drip[7] = [lambda: q_chain(2, 0)]
        drip[8] = [lambda: q_chain(2, 1)]
        drip[9] = [lambda: q_chain(2, 2)]
        drip[10] = [lambda: q_chain(2, 3)] + [
            lambda mt=mt: oproj_chain(0, mt) for mt in range(C // P)
        ]
        drip[11] = [lambda mt=mt: oproj_chain(1, mt) for mt in range(C // P)] + [
            lambda mt=mt: oproj_chain(2, mt) for mt in range(C // P)
        ]
        for th in upfront:
            th()

        # ---- attention pipeline: flat cross-iteration software pipeline.
        # Slots are (it, kt); QK+exp lead, AV trails AV_LAG slots behind
        # (crossing iteration boundaries), drains are emitted as soon as an
        # iteration's last AV is emitted — mid-next-iteration — so the
        # 2-deep x_ps ring never stalls the PE.
        from collections import deque

        AV_LAG = 4
        rows = (slice(0, DH), slice(DH, 2 * DH))
        x_tiles = {}
        p_tiles = {}
        av_q = deque()

        def emit_av(it, kt):
            p3 = it // 4
            pt = p_tiles.pop((it, kt))
            for hh in range(2):
                nc.tensor.matmul(
                    x_tiles[it][hh],
                    v_s[:, kt, 2 * p3 + hh, :],
                    pt[:, hh * QCH : (hh + 1) * QCH],
                    start=(kt == 0),
                    stop=(kt == NKT - 1),
                )

        def emit_drain(it):
            p3, j4 = divmod(it, 4)
            qsl = slice(j4 * QCH, (j4 + 1) * QCH)
            for hh in range(2):
                # free the PSUM accumulator immediately: body + denom to SBUF
                xs = xsb.tile([DH, QCH], f32, tag="xs")
                nc.vector.tensor_copy(xs, x_tiles[it][hh][0:DH, :])
                r0 = small.tile([1, QCH], f32, tag="r0")
                nc.vector.tensor_copy(r0, x_tiles[it][hh][DH : DH + 1, :])
                r = small.tile([1, QCH], f32, tag="r")
                nc.vector.reciprocal_approx_fast(r, r0)
                rd = drp.tile([1, QCH], f32, tag="rd")
                nc.gpsimd.dma_start(out=rd, in_=r)
                rb = brd.tile([DH, QCH], f32, tag="rb")
                nc.gpsimd.dma_start(out=rb, in_=rd.to_broadcast([DH, QCH]))
                if hh == 0:
                    nc.vector.tensor_mul(xn_t[p3][0:DH, qsl], xs, rb)
                else:
                    tmp = brd.tile([DH, QCH], bf16, tag="tmp")
                    nc.vector.tensor_mul(tmp, xs, rb)
                    nc.sync.dma_start(out=xn_t[p3][DH : 2 * DH, qsl], in_=tmp)
            del x_tiles[it]

        todo = []
        for it in range(12):
            p3, j4 = divmod(it, 4)
            qsl = slice(j4 * QCH, (j4 + 1) * QCH)
            x_tiles[it] = [
                x_ps.tile([VW, QCH], f32, tag="x", name=f"x_{it}_{hh}")
                for hh in range(2)
            ]
            todo.extend(drip[it])
            for kt in range(NKT):
                # trailing AV first: its deps are oldest
                if len(av_q) >= AV_LAG:
                    a_it, a_kt = av_q.popleft()
                    emit_av(a_it, a_kt)
                    if a_kt == NKT - 1:
                        emit_drain(a_it)
                lt = lt_ps.tile([P, 2 * QCH], f32, tag="lt")
                for hh in range(2):
                    nc.tensor.matmul(
                        lt[:, hh * QCH : (hh + 1) * QCH],
                        kT_t[p3][rows[hh], kt * P : (kt + 1) * P],
                        qT_t[p3][rows[hh], qsl],
                        start=True,
                        stop=True,
                    )
                pt = pp.tile([P, 2 * QCH], bf16, tag="p")
                nc.scalar.activation(pt, lt, EXP)
                p_tiles[(it, kt)] = pt
                av_q.append((it, kt))
                if todo:
                    todo.pop(0)()
        while av_q:
            a_it, a_kt = av_q.popleft()
            emit_av(a_it, a_kt)
            if a_kt == NKT - 1:
                emit_drain(a_it)
        while todo:
            todo.pop(0)()
        # final chunk's output projection
        for mt in range(C // P):
            oproj_chain(3, mt)

    nc.finalize()
    return nc


def _get_prog():
    global _prog
    if _prog is None:
        _prog = _build()
    return _prog


def _shard_inputs(query, key, value, Wq, Wk, Wv, Wo):
    from ml_dtypes import bfloat16

    in_maps = []
    for core in range(8):
        b, hg = core // 2, core % 2
        sl = slice(hg * HB, (hg + 1) * HB)
        in_maps.append(
            {
                "xq": np.ascontiguousarray(query[b].T).astype(bfloat16),
                "xk": np.ascontiguousarray(key[b].T).astype(bfloat16),
                "xv": np.ascontiguousarray(value[b].T).astype(bfloat16),
                "wq": np.ascontiguousarray((Wq[sl, :] * SCALE).T).astype(bfloat16),
                "wk": np.ascontiguousarray(Wk[sl, :].T).astype(bfloat16),
                "wv": np.ascontiguousarray(Wv[sl, :].T).astype(bfloat16),
                "wo": np.ascontiguousarray(Wo[:, sl].T).astype(bfloat16),
            }
        )
    return in_maps


def kernel(query, key, value, Wq, Wk, Wv, Wo, bo):
    query, key, value = np.asarray(query), np.asarray(key), np.asarray(value)
    Wq, Wk, Wv, Wo = np.asarray(Wq), np.asarray(Wk), np.asarray(Wv), np.asarray(Wo)
    bo = np.asarray(bo).astype(np.float32)

    from concourse.bass_utils import run_bass_kernel_spmd

    nc = _get_prog()
    in_maps = _shard_inputs(query, key, value, Wq, Wk, Wv, Wo)
    res = run_bass_kernel_spmd(nc, in_maps, list(range(8))).results

    out = np.empty((B, NQ, C), np.float32)
    for b in range(B):
        acc = res[2 * b]["out"].astype(np.float32) + res[2 * b + 1]["out"].astype(
            np.float32
        )
        out[b] = acc.T + bo[None, :]
    return out
